# revision 3
# baseline (speedup 1.0000x reference)
"""Dcls1d via overlap-save rFFT conv on 8 Trainium2 NeuronCores.

Replaces the dense-56 direct conv (3.67M PE cycles/core) with an F=256
overlap-save FFT convolution (~420k PE cycles/core):
  fwd:  per segment (21/batch, V=201 valid outs), DFT as 2-chain matmuls
        with stationary = host-pre-transposed x segment [t, c], moving =
        packed rFFT matrix [256t x 256 cols: Re k=0..128 | Im k=1..127]
        -> x_hat[c, bins] in SBUF (bf16).
  pw:   per bin k, complex pointwise mult-accumulate over c as matmuls:
        stationary = x_hat slices [c, S], moving = streamed D_hat[k]
        [c, o] bf16 from DRAM; psum [S, o]; corner-turn scatter into
        Oh[k-part, S, o] via SBUF->SBUF DMA.
  inv:  per segment, stationary = Oh slices [binsri, o], moving = packed
        irFFT matrix [256 x 201] (1/F and alpha_k baked in) -> y[o, t]
        directly, + bias, DMA out.

Sharding: data-parallel over batch (4 per core), weights spectra
broadcast. Host precomputes D_hat = conj(rfft(D_dense, 256)) in bf16.
"""

import numpy as np
import ml_dtypes
from contextlib import ExitStack

import concourse.bacc as bacc
import concourse.mybir as mybir
import concourse.tile as tile
from concourse import masks  # noqa: F401  (kept for parity with baseline env)
from concourse.bass_utils import run_bass_kernel_spmd

DT = mybir.dt
BF = ml_dtypes.bfloat16

B, CIN, COUT, L = 32, 256, 256, 4096
KTAPS, DIL, PAD = 7, 8, 28
LD = KTAPS * DIL  # 56
TOUT = L + 1  # 4097
NCORES = 8
BPC = B // NCORES  # 4

F = 256
NBIN = F // 2 + 1  # 129
V = F - LD + 1  # 201
SEGS = (TOUT + V - 1) // V  # 21
YL = SEGS * V  # 4221
LPAD = V * (SEGS - 1) + F  # 4276

_nc_cache = {}


def build_dense_kernel(weight: np.ndarray, P: np.ndarray) -> np.ndarray:
    """Scatter taps into dense [O, C, LD] kernel (fp32-exact vs reference)."""
    w = weight.astype(np.float32)
    pos = np.clip(P.astype(np.float32) + np.float32(LD // 2), np.float32(0.0), np.float32(LD - 1))
    lo = np.floor(pos)
    frac = pos - lo
    lo_i = lo.astype(np.int64)
    hi_i = np.minimum(lo_i + 1, LD - 1)
    O, C, K = w.shape
    oi = np.arange(O)[:, None, None]
    ci = np.arange(C)[None, :, None]
    D = np.zeros((O, C, LD), np.float32)
    np.add.at(D, (oi, ci, lo_i), w * (np.float32(1.0) - frac))
    np.add.at(D, (oi, ci, hi_i), w * frac)
    return D


def build_consts(D):
    """Wf [256,256], WI [256,V], Dh [129,2,C,O] (fp32; cast at use)."""
    t = np.arange(F)[:, None]
    k = np.arange(NBIN)[None, :]
    ang = 2 * np.pi * t * k / F
    Wf = np.concatenate([np.cos(ang), -np.sin(ang[:, 1:128])], axis=1)
    m = np.arange(V)[None, :]
    kk = np.arange(NBIN)[:, None]
    alpha = np.where((kk == 0) | (kk == NBIN - 1), 1.0, 2.0) / F
    angi = 2 * np.pi * kk * m / F
    WI = np.concatenate([alpha * np.cos(angi), -(alpha * np.sin(angi))[1:128]], axis=0)
    Kh = np.conj(np.fft.rfft(D, n=F, axis=2))  # [O,C,129]
    Dh = np.stack([Kh.real, Kh.imag], axis=0)  # [2,O,C,129]
    Dh = np.ascontiguousarray(np.transpose(Dh, (3, 0, 2, 1)))  # [129,2,C,O]
    return Wf.astype(np.float32), WI.astype(np.float32), Dh.astype(np.float32)


def build_nc(bpc=BPC):
    S = bpc * SEGS
    nc = bacc.Bacc("TRN2", target_bir_lowering=False, debug=False)
    xt_d = nc.dram_tensor("xt", [bpc, SEGS, F, CIN], DT.bfloat16, kind="ExternalInput").ap()
    dh_d = nc.dram_tensor("dh", [NBIN, 2, 2, 128, COUT], DT.bfloat16, kind="ExternalInput").ap()
    wf_d = nc.dram_tensor("wf", [2, 128, 256], DT.bfloat16, kind="ExternalInput").ap()
    wi_d = nc.dram_tensor("wi", [2, 128, V], DT.bfloat16, kind="ExternalInput").ap()
    bias_d = nc.dram_tensor("bias", [128, 2], DT.float32, kind="ExternalInput").ap()
    y_d = nc.dram_tensor("y", [bpc, COUT, YL], DT.float32, kind="ExternalOutput").ap()

    with ExitStack() as ctx:
        tc = ctx.enter_context(tile.TileContext(nc))
        cpool = ctx.enter_context(tc.tile_pool(name="c", bufs=1))
        xpool = ctx.enter_context(tc.tile_pool(name="x", bufs=3))
        dhpool = ctx.enter_context(tc.tile_pool(name="dh", bufs=6))
        ngpool = ctx.enter_context(tc.tile_pool(name="ng", bufs=3))
        stpool = ctx.enter_context(tc.tile_pool(name="st", bufs=4))
        ypool = ctx.enter_context(tc.tile_pool(name="y", bufs=4))
        psF = ctx.enter_context(tc.tile_pool(name="psF", bufs=2, space="PSUM"))
        psPW = ctx.enter_context(tc.tile_pool(name="psPW", bufs=2, space="PSUM"))
        psI = ctx.enter_context(tc.tile_pool(name="psI", bufs=2, space="PSUM"))

        wft = cpool.tile([128, 2, 256], DT.bfloat16)
        wit = cpool.tile([128, 2, V], DT.bfloat16)
        biast = cpool.tile([128, 2], DT.float32)
        for tcn in range(2):
            nc.scalar.dma_start(wft[:, tcn, :], wf_d[tcn])
            nc.scalar.dma_start(wit[:, tcn, :], wi_d[tcn])
        nc.scalar.dma_start(biast[:], bias_d[:])

        # persistent SBUF stores
        xh = cpool.tile([128, 2, 256, S], DT.bfloat16, name="xh", tag="xh")
        ohA = cpool.tile([128, S, 256], DT.bfloat16, name="ohA", tag="ohA")
        ohB = cpool.tile([128, S, 256], DT.bfloat16, name="ohB", tag="ohB")

        # ---- forward DFT ----
        for b in range(bpc):
            for i in range(SEGS):
                s = b * SEGS + i
                xs = xpool.tile([128, 2, CIN], DT.bfloat16)
                for tcn in range(2):
                    nc.scalar.dma_start(
                        xs[:, tcn, :], xt_d[b, i, tcn * 128 : (tcn + 1) * 128, :]
                    )
                for cb in range(2):
                    pf = psF.tile([128, 256], DT.float32)
                    for tcn in range(2):
                        nc.tensor.matmul(
                            pf[:],
                            xs[:, tcn, cb * 128 : (cb + 1) * 128],
                            wft[:, tcn, :],
                            start=(tcn == 0),
                            stop=(tcn == 1),
                        )
                    nc.vector.tensor_copy(xh[:, cb, :, s], pf[:])

        # ---- pointwise complex multiply per bin ----
        for k in range(NBIN):
            dht = dhpool.tile([128, 2, 2, COUT], DT.bfloat16)
            for ri in range(2):
                for cb in range(2):
                    nc.sync.dma_start(dht[:, ri, cb, :], dh_d[k, ri, cb])
            has_im = 0 < k < NBIN - 1
            if has_im:
                ng = ngpool.tile([128, 2, S], DT.bfloat16)
                for cb in range(2):
                    nc.vector.tensor_scalar_mul(
                        ng[:, cb, :], xh[:, cb, 128 + k, :], -1.0
                    )
            # Re: sum_c Xr*Dr + (-Xi)*Di
            pre = psPW.tile([S, 256], DT.float32)
            n_acc = 4 if has_im else 2
            idx = 0
            for cb in range(2):
                nc.tensor.matmul(
                    pre[:], xh[:, cb, k, :], dht[:, 0, cb, :],
                    start=(idx == 0), stop=(idx == n_acc - 1),
                )
                idx += 1
            if has_im:
                for cb in range(2):
                    nc.tensor.matmul(
                        pre[:], ng[:, cb, :], dht[:, 1, cb, :],
                        start=False, stop=(idx == n_acc - 1),
                    )
                    idx += 1
            stre = stpool.tile([S, 256], DT.bfloat16)
            nc.vector.tensor_copy(stre[:], pre[:])
            dst = ohA[k : k + 1] if k < 128 else ohB[0:1]
            nc.gpsimd.dma_start(dst, stre[:])
            # Im: sum_c Xr*Di + Xi*Dr  (k = 1..127)
            if has_im:
                pim = psPW.tile([S, 256], DT.float32)
                idx = 0
                for cb in range(2):
                    nc.tensor.matmul(
                        pim[:], xh[:, cb, k, :], dht[:, 1, cb, :],
                        start=(idx == 0), stop=False,
                    )
                    idx += 1
                for cb in range(2):
                    nc.tensor.matmul(
                        pim[:], xh[:, cb, 128 + k, :], dht[:, 0, cb, :],
                        start=False, stop=(idx == 3),
                    )
                    idx += 1
                stim = stpool.tile([S, 256], DT.bfloat16)
                nc.vector.tensor_copy(stim[:], pim[:])
                nc.gpsimd.dma_start(ohB[k : k + 1], stim[:])

        # ---- inverse DFT + bias ----
        for s in range(S):
            b, i = divmod(s, SEGS)
            for ot in range(2):
                pv = psI.tile([128, V], DT.float32)
                nc.tensor.matmul(
                    pv[:], ohA[:, s, ot * 128 : (ot + 1) * 128], wit[:, 0, :],
                    start=True, stop=False,
                )
                nc.tensor.matmul(
                    pv[:], ohB[:, s, ot * 128 : (ot + 1) * 128], wit[:, 1, :],
                    start=False, stop=True,
                )
                yo = ypool.tile([128, V], DT.float32)
                nc.vector.tensor_scalar_add(yo[:], pv[:], biast[:, ot : ot + 1])
                nc.gpsimd.dma_start(
                    y_d[b, ot * 128 : (ot + 1) * 128, i * V : (i + 1) * V], yo[:]
                )

    nc.compile()
    return nc


def host_inputs(input, weight, P, bias):
    """Host-side staging: xt segments (transposed, bf16) + spectra consts."""
    D = build_dense_kernel(weight, P)
    Wf, WI, Dh = build_consts(D)
    wf = np.ascontiguousarray(Wf.reshape(2, 128, 256)).astype(BF)
    wi = np.ascontiguousarray(WI.reshape(2, 128, V)).astype(BF)
    dh = np.ascontiguousarray(Dh.reshape(NBIN, 2, 2, 128, COUT)).astype(BF)
    bias2 = np.ascontiguousarray(np.asarray(bias, np.float32).reshape(2, 128).T)
    xpad = np.zeros((input.shape[0], CIN, LPAD), np.float32)
    xpad[:, :, PAD : PAD + L] = input
    xpad = xpad.astype(BF)
    idx = V * np.arange(SEGS)[:, None] + np.arange(F)[None, :]
    segs = xpad[:, :, idx]  # [B, C, SEGS, F]
    xt = np.ascontiguousarray(segs.transpose(0, 2, 3, 1))  # [B, SEGS, F, C]
    return xt, dh, wf, wi, bias2


def make_in_maps(inputs):
    xt, dh, wf, wi, bias2 = host_inputs(
        np.ascontiguousarray(inputs["input"], np.float32),
        inputs["weight"],
        inputs["P"],
        inputs["bias"],
    )
    return [
        {
            "xt": np.ascontiguousarray(xt[i * BPC : (i + 1) * BPC]),
            "dh": dh,
            "wf": wf,
            "wi": wi,
            "bias": bias2,
        }
        for i in range(NCORES)
    ]


def kernel(input, weight, P, bias):
    if "nc" not in _nc_cache:
        _nc_cache["nc"] = build_nc()
    nc = _nc_cache["nc"]
    in_maps = make_in_maps(
        {"input": input, "weight": weight, "P": P, "bias": bias}
    )
    res = run_bass_kernel_spmd(nc, in_maps, core_ids=list(range(NCORES)))
    out = np.concatenate([r["y"] for r in res.results], axis=0)
    return np.ascontiguousarray(out[:, :, :TOUT])


# revision 14
# speedup vs baseline: 1.1124x; 1.1124x over previous
"""Dcls1d via overlap-save rFFT conv on 8 Trainium2 NeuronCores.

Replaces the dense-56 direct conv (3.67M PE cycles/core) with an F=256
overlap-save FFT convolution (~420k PE cycles/core):
  fwd:  per segment (21/batch, V=201 valid outs), DFT as 2-chain matmuls
        with stationary = host-pre-transposed x segment [t, c], moving =
        packed rFFT matrix [256t x 256 cols: Re k=0..128 | Im k=1..127]
        -> x_hat[c, bins] in SBUF (bf16).
  pw:   per bin k, complex pointwise mult-accumulate over c as matmuls:
        stationary = x_hat slices [c, S], moving = streamed D_hat[k]
        [c, o] bf16 from DRAM (one 256KB DMA per bin); psum [S, 2, 256]
        holds Re and Im; drains stage into wide 8-row buffers that flush
        with one corner-turn SBUF->SBUF DMA into Oh[k-part, S, o].
  inv:  per segment, stationary = Oh slices [binsri, o], moving = packed
        irFFT matrix [256 x 201] (1/F and alpha_k baked in) -> y[o, t]
        directly, + bias, one merged DMA out per segment.

Sharding: data-parallel over batch (4 per core), weights spectra
broadcast. Host precomputes D_hat = conj(rfft(D_dense, 256)) in bf16.
"""

import numpy as np
import ml_dtypes
from contextlib import ExitStack

import concourse.bacc as bacc
import concourse.mybir as mybir
import concourse.tile as tile
from concourse.bass_utils import run_bass_kernel_spmd

DT = mybir.dt
BF = ml_dtypes.bfloat16

B, CIN, COUT, L = 32, 256, 256, 4096
KTAPS, DIL, PAD = 7, 8, 28
LD = KTAPS * DIL  # 56
TOUT = L + 1  # 4097
NCORES = 8
BPC = B // NCORES  # 4

F = 256
NBIN = F // 2 + 1  # 129
V = F - LD + 1  # 201
SEGS = (TOUT + V - 1) // V  # 21
YL = SEGS * V  # 4221
LPAD = V * (SEGS - 1) + F  # 4276

_nc_cache = {}


def build_dense_kernel(weight: np.ndarray, P: np.ndarray) -> np.ndarray:
    """Scatter taps into dense [O, C, LD] kernel (fp32-exact vs reference)."""
    w = weight.astype(np.float32)
    pos = np.clip(P.astype(np.float32) + np.float32(LD // 2), np.float32(0.0), np.float32(LD - 1))
    lo = np.floor(pos)
    frac = pos - lo
    lo_i = lo.astype(np.int64)
    hi_i = np.minimum(lo_i + 1, LD - 1)
    O, C, K = w.shape
    oi = np.arange(O)[:, None, None]
    ci = np.arange(C)[None, :, None]
    D = np.zeros((O, C, LD), np.float32)
    np.add.at(D, (oi, ci, lo_i), w * (np.float32(1.0) - frac))
    np.add.at(D, (oi, ci, hi_i), w * frac)
    return D


def build_consts(D):
    """Wf [256,256], WI [256,V], Dh [129,2,C,O] (fp32; cast at use)."""
    t = np.arange(F)[:, None]
    k = np.arange(NBIN)[None, :]
    ang = 2 * np.pi * t * k / F
    Wf = np.concatenate([np.cos(ang), -np.sin(ang[:, 1:128])], axis=1)
    m = np.arange(V)[None, :]
    kk = np.arange(NBIN)[:, None]
    alpha = np.where((kk == 0) | (kk == NBIN - 1), 1.0, 2.0) / F
    angi = 2 * np.pi * kk * m / F
    WI = np.concatenate([alpha * np.cos(angi), -(alpha * np.sin(angi))[1:128]], axis=0)
    Kh = np.conj(np.fft.rfft(D, n=F, axis=2))  # [O,C,129]
    Dh = np.stack([Kh.real, Kh.imag], axis=0)  # [2,O,C,129]
    Dh = np.ascontiguousarray(np.transpose(Dh, (3, 0, 2, 1)))  # [129,2,C,O]
    return Wf.astype(np.float32), WI.astype(np.float32), Dh.astype(np.float32)


def build_nc(bpc=BPC):
    S = bpc * SEGS
    nc = bacc.Bacc("TRN2", target_bir_lowering=False, debug=False)
    xt_d = nc.dram_tensor("xt", [bpc, SEGS, 2, 128, CIN], DT.bfloat16, kind="ExternalInput").ap()
    # dh[k, cp, cb, ri, o]
    dh_d = nc.dram_tensor("dh", [NBIN, 128, 2, 2, COUT], DT.bfloat16, kind="ExternalInput").ap()
    wf_d = nc.dram_tensor("wf", [2, 128, 256], DT.bfloat16, kind="ExternalInput").ap()
    wi_d = nc.dram_tensor("wi", [2, 128, V], DT.bfloat16, kind="ExternalInput").ap()
    bias_d = nc.dram_tensor("bias", [128, 2], DT.float32, kind="ExternalInput").ap()
    y_d = nc.dram_tensor("y", [bpc, 2, 128, YL], DT.float32, kind="ExternalOutput").ap()

    with ExitStack() as ctx:
        tc = ctx.enter_context(tile.TileContext(nc))
        cpool = ctx.enter_context(tc.tile_pool(name="c", bufs=1))
        xpool = ctx.enter_context(tc.tile_pool(name="x", bufs=3))
        dhpool = ctx.enter_context(tc.tile_pool(name="dh", bufs=4))
        ngpool = ctx.enter_context(tc.tile_pool(name="ng", bufs=3))
        stpool = ctx.enter_context(tc.tile_pool(name="st", bufs=3))
        ypool = ctx.enter_context(tc.tile_pool(name="y", bufs=4))
        psF = ctx.enter_context(tc.tile_pool(name="psF", bufs=2, space="PSUM"))
        psPW = ctx.enter_context(tc.tile_pool(name="psPW", bufs=2, space="PSUM"))
        psI = ctx.enter_context(tc.tile_pool(name="psI", bufs=2, space="PSUM"))

        wft = cpool.tile([128, 2, 256], DT.bfloat16)
        wit = cpool.tile([128, 2, V], DT.bfloat16)
        biast = cpool.tile([128, 2], DT.float32)
        for tcn in range(2):
            nc.scalar.dma_start(wft[:, tcn, :], wf_d[tcn])
            nc.scalar.dma_start(wit[:, tcn, :], wi_d[tcn])
        nc.scalar.dma_start(biast[:], bias_d[:])

        # persistent SBUF stores
        xh = cpool.tile([128, 2, 256, S], DT.bfloat16, name="xh", tag="xh")
        ohA = cpool.tile([128, S, 256], DT.bfloat16, name="ohA", tag="ohA")
        ohB = cpool.tile([128, S, 256], DT.bfloat16, name="ohB", tag="ohB")

        # ---- forward DFT ----
        for b in range(bpc):
            for i in range(SEGS):
                s = b * SEGS + i
                xs = xpool.tile([128, 2, CIN], DT.bfloat16)
                nc.scalar.dma_start(xs[:], xt_d[b, i].transpose([1, 0, 2]))
                pf = psF.tile([128, 2, 256], DT.float32)
                for cb in range(2):
                    for tcn in range(2):
                        nc.tensor.matmul(
                            pf[:, cb, :],
                            xs[:, tcn, cb * 128 : (cb + 1) * 128],
                            wft[:, tcn, :],
                            start=(tcn == 0),
                            stop=(tcn == 1),
                            skip_group_check=True,
                        )
                if s % 2 == 0:
                    nc.vector.tensor_copy(xh[:, :, :, s], pf[:])
                else:
                    nc.scalar.copy(xh[:, :, :, s], pf[:])

        # ---- pointwise complex multiply ----
        # jobs per bin k: Re chain (always) into psum col 0, Im chain
        # (k=1..127) into col 1. Drains go into wide 8-row staging tiles
        # (one for ohA rows = Re k, one for ohB rows = Im k / Re 128),
        # flushed by single corner-turn DMAs.
        GRP = 4
        stA = stB = None
        stA_rows = stB_rows = None

        def flush(st, rows, oh):
            if st is None or not rows:
                return
            # partition dim must stay outermost in SBUF APs, so scatter
            # row-by-row: dst [1, S, 256] <- src [S, 1, 256]
            for j, r in enumerate(rows):
                nc.gpsimd.dma_start(oh[r : r + 1], st[:, j : j + 1, :])

        for k in range(NBIN):
            dht = dhpool.tile([128, 2, 2, COUT], DT.bfloat16)
            nc.sync.dma_start(dht[:], dh_d[k])
            has_im = 0 < k < NBIN - 1
            if has_im:
                ng = ngpool.tile([128, 2, S], DT.bfloat16)
                for cb in range(2):
                    nc.vector.tensor_scalar_mul(
                        ng[:, cb, :], xh[:, cb, 128 + k, :], -1.0
                    )
            ps = psPW.tile([S, 2, 256], DT.float32)
            # Re: Xr*Dr + (-Xi)*Di
            n_acc = 4 if has_im else 2
            idx = 0
            for cb in range(2):
                nc.tensor.matmul(
                    ps[:, 0, :], xh[:, cb, k, :], dht[:, cb, 0, :],
                    start=(idx == 0), stop=(idx == n_acc - 1),
                    skip_group_check=True,
                )
                idx += 1
            if has_im:
                for cb in range(2):
                    nc.tensor.matmul(
                        ps[:, 0, :], ng[:, cb, :], dht[:, cb, 1, :],
                        start=False, stop=(idx == n_acc - 1),
                        skip_group_check=True,
                    )
                    idx += 1
                # Im: Xr*Di + Xi*Dr
                idx = 0
                for cb in range(2):
                    nc.tensor.matmul(
                        ps[:, 1, :], xh[:, cb, k, :], dht[:, cb, 1, :],
                        start=(idx == 0), stop=False,
                        skip_group_check=True,
                    )
                    idx += 1
                for cb in range(2):
                    nc.tensor.matmul(
                        ps[:, 1, :], xh[:, cb, 128 + k, :], dht[:, cb, 0, :],
                        start=False, stop=(idx == 3),
                        skip_group_check=True,
                    )
                    idx += 1
            # stage Re row (ohA row k, or ohB row 0 for k=128)
            def stage(dst, src, use_act):
                if use_act:
                    nc.scalar.copy(dst, src)
                else:
                    nc.vector.tensor_copy(dst, src)

            if k < 128:
                if stA is None:
                    stA = stpool.tile([S, GRP, 256], DT.bfloat16)
                    stA_rows = []
                stage(stA[:, len(stA_rows), :], ps[:, 0, :], k % 2)
                stA_rows.append(k)
                if len(stA_rows) == GRP:
                    flush(stA, stA_rows, ohA)
                    stA = None
            else:
                stx = stpool.tile([S, 1, 256], DT.bfloat16)
                stage(stx[:, 0, :], ps[:, 0, :], k % 2)
                flush(stx, [0], ohB)
            # stage Im row (ohB row k)
            if has_im:
                if stB is None:
                    stB = stpool.tile([S, GRP, 256], DT.bfloat16)
                    stB_rows = []
                stage(stB[:, len(stB_rows), :], ps[:, 1, :], (k + 1) % 2)
                stB_rows.append(k)
                if len(stB_rows) == GRP:
                    flush(stB, stB_rows, ohB)
                    stB = None
        flush(stA, stA_rows, ohA)
        flush(stB, stB_rows, ohB)

        # ---- inverse DFT + bias ----
        for s in range(S):
            b, i = divmod(s, SEGS)
            pv = psI.tile([128, 2, V], DT.float32)
            for ot in range(2):
                nc.tensor.matmul(
                    pv[:, ot, :], ohA[:, s, ot * 128 : (ot + 1) * 128], wit[:, 0, :],
                    start=True, stop=False, skip_group_check=True,
                )
                nc.tensor.matmul(
                    pv[:, ot, :], ohB[:, s, ot * 128 : (ot + 1) * 128], wit[:, 1, :],
                    start=False, stop=True, skip_group_check=True,
                )
            yo = ypool.tile([128, 2, V], DT.float32)
            for ot in range(2):
                if s % 2 == 0:
                    nc.vector.tensor_scalar_add(
                        yo[:, ot, :], pv[:, ot, :], biast[:, ot : ot + 1]
                    )
                else:
                    nc.scalar.add(yo[:, ot, :], pv[:, ot, :], biast[:, ot : ot + 1])
            nc.scalar.dma_start(
                y_d[b, :, :, i * V : (i + 1) * V].transpose([1, 0, 2]), yo[:]
            )

    nc.compile()
    return nc


def host_inputs(input, weight, P, bias):
    """Host-side staging: xt segments (transposed, bf16) + spectra consts."""
    D = build_dense_kernel(weight, P)
    Wf, WI, Dh = build_consts(D)
    wf = np.ascontiguousarray(Wf.reshape(2, 128, 256)).astype(BF)
    wi = np.ascontiguousarray(WI.reshape(2, 128, V)).astype(BF)
    # Dh [129, 2ri, C, O] -> dh[k, cp, cb, ri, o]
    dh = np.ascontiguousarray(
        Dh.reshape(NBIN, 2, 2, 128, COUT).transpose(0, 3, 2, 1, 4)
    ).astype(BF)
    bias2 = np.ascontiguousarray(np.asarray(bias, np.float32).reshape(2, 128).T)
    xpad = np.zeros((input.shape[0], CIN, LPAD), np.float32)
    xpad[:, :, PAD : PAD + L] = input
    xpad = xpad.astype(BF)
    idx = V * np.arange(SEGS)[:, None] + np.arange(F)[None, :]
    segs = xpad[:, :, idx]  # [B, C, SEGS, F]
    xt = np.ascontiguousarray(segs.transpose(0, 2, 3, 1))  # [B, SEGS, F, C]
    xt = xt.reshape(input.shape[0], SEGS, 2, 128, CIN)
    return xt, dh, wf, wi, bias2


def make_in_maps(inputs):
    xt, dh, wf, wi, bias2 = host_inputs(
        np.ascontiguousarray(inputs["input"], np.float32),
        inputs["weight"],
        inputs["P"],
        inputs["bias"],
    )
    return [
        {
            "xt": np.ascontiguousarray(xt[i * BPC : (i + 1) * BPC]),
            "dh": dh,
            "wf": wf,
            "wi": wi,
            "bias": bias2,
        }
        for i in range(NCORES)
    ]


def kernel(input, weight, P, bias):
    if "nc" not in _nc_cache:
        _nc_cache["nc"] = build_nc()
    nc = _nc_cache["nc"]
    in_maps = make_in_maps(
        {"input": input, "weight": weight, "P": P, "bias": bias}
    )
    res = run_bass_kernel_spmd(nc, in_maps, core_ids=list(range(NCORES)))
    out = np.concatenate(
        [r["y"].reshape(BPC, COUT, YL) for r in res.results], axis=0
    )
    return np.ascontiguousarray(out[:, :, :TOUT])


# revision 17
# speedup vs baseline: 1.1376x; 1.0227x over previous
"""Dcls1d via overlap-save rFFT conv on 8 Trainium2 NeuronCores.

Replaces the dense-56 direct conv (3.67M PE cycles/core) with an F=256
overlap-save FFT convolution (~420k PE cycles/core):
  fwd:  per segment (21/batch, V=201 valid outs), DFT as 2-chain matmuls
        with stationary = host-pre-transposed x segment [t, c], moving =
        packed rFFT matrix [256t x 256 cols: Re k=0..128 | Im k=1..127]
        -> x_hat[c, bins] in SBUF (bf16).
  pw:   per bin k, complex pointwise mult-accumulate over c as matmuls:
        stationary = x_hat slices [c, S], moving = streamed D_hat[k]
        [c, o] bf16 from DRAM (one 256KB DMA per bin); psum [S, 2, 256]
        holds Re and Im; drains stage into wide 8-row buffers that flush
        with one corner-turn SBUF->SBUF DMA into Oh[k-part, S, o].
  inv:  per segment, stationary = Oh slices [binsri, o], moving = packed
        irFFT matrix [256 x 201] (1/F and alpha_k baked in) -> y[o, t]
        directly, + bias, one merged DMA out per segment.

Sharding: data-parallel over batch (4 per core), weights spectra
broadcast. Host precomputes D_hat = conj(rfft(D_dense, 256)) in bf16.
"""

import numpy as np
import ml_dtypes
from contextlib import ExitStack

import concourse.bacc as bacc
import concourse.mybir as mybir
import concourse.tile as tile
from concourse.bass_utils import run_bass_kernel_spmd

DT = mybir.dt
BF = ml_dtypes.bfloat16

B, CIN, COUT, L = 32, 256, 256, 4096
KTAPS, DIL, PAD = 7, 8, 28
LD = KTAPS * DIL  # 56
TOUT = L + 1  # 4097
NCORES = 8
BPC = B // NCORES  # 4

F = 256
NBIN = F // 2 + 1  # 129
V = F - LD + 1  # 201
SEGS = (TOUT + V - 1) // V  # 21
YL = SEGS * V  # 4221
LPAD = V * (SEGS - 1) + F  # 4276

_nc_cache = {}


def build_dense_kernel(weight: np.ndarray, P: np.ndarray) -> np.ndarray:
    """Scatter taps into dense [O, C, LD] kernel (fp32-exact vs reference)."""
    w = weight.astype(np.float32)
    pos = np.clip(P.astype(np.float32) + np.float32(LD // 2), np.float32(0.0), np.float32(LD - 1))
    lo = np.floor(pos)
    frac = pos - lo
    lo_i = lo.astype(np.int64)
    hi_i = np.minimum(lo_i + 1, LD - 1)
    O, C, K = w.shape
    oi = np.arange(O)[:, None, None]
    ci = np.arange(C)[None, :, None]
    D = np.zeros((O, C, LD), np.float32)
    np.add.at(D, (oi, ci, lo_i), w * (np.float32(1.0) - frac))
    np.add.at(D, (oi, ci, hi_i), w * frac)
    return D


def build_consts(D):
    """Wf [256,256], WI [256,V], Dh [129,2,C,O] (fp32; cast at use)."""
    t = np.arange(F)[:, None]
    k = np.arange(NBIN)[None, :]
    ang = 2 * np.pi * t * k / F
    Wf = np.concatenate([np.cos(ang), -np.sin(ang[:, 1:128])], axis=1)
    m = np.arange(V)[None, :]
    kk = np.arange(NBIN)[:, None]
    alpha = np.where((kk == 0) | (kk == NBIN - 1), 1.0, 2.0) / F
    angi = 2 * np.pi * kk * m / F
    WI = np.concatenate([alpha * np.cos(angi), -(alpha * np.sin(angi))[1:128]], axis=0)
    Kh = np.conj(np.fft.rfft(D, n=F, axis=2))  # [O,C,129]
    Dh = np.stack([Kh.real, Kh.imag], axis=0)  # [2,O,C,129]
    Dh = np.ascontiguousarray(np.transpose(Dh, (3, 0, 2, 1)))  # [129,2,C,O]
    return Wf.astype(np.float32), WI.astype(np.float32), Dh.astype(np.float32)


def build_nc(bpc=BPC):
    S = bpc * SEGS
    nc = bacc.Bacc("TRN2", target_bir_lowering=False, debug=False)
    xt_d = nc.dram_tensor("xt", [bpc, SEGS, 2, 128, CIN], DT.bfloat16, kind="ExternalInput").ap()
    # dh[k, cp, cb, ri, o]
    dh_d = nc.dram_tensor("dh", [NBIN, 128, 2, 2, COUT], DT.bfloat16, kind="ExternalInput").ap()
    wf_d = nc.dram_tensor("wf", [2, 128, 256], DT.bfloat16, kind="ExternalInput").ap()
    wi_d = nc.dram_tensor("wi", [2, 128, V], DT.bfloat16, kind="ExternalInput").ap()
    bias_d = nc.dram_tensor("bias", [128, 2], DT.float32, kind="ExternalInput").ap()
    y_d = nc.dram_tensor("y", [bpc, 2, 128, YL], DT.float32, kind="ExternalOutput").ap()

    with ExitStack() as ctx:
        tc = ctx.enter_context(tile.TileContext(nc))
        cpool = ctx.enter_context(tc.tile_pool(name="c", bufs=1))
        xpool = ctx.enter_context(tc.tile_pool(name="x", bufs=3))
        dhpool = ctx.enter_context(tc.tile_pool(name="dh", bufs=6))
        ngpool = ctx.enter_context(tc.tile_pool(name="ng", bufs=3))
        stpool = ctx.enter_context(tc.tile_pool(name="st", bufs=3))
        ypool = ctx.enter_context(tc.tile_pool(name="y", bufs=4))
        psF = ctx.enter_context(tc.tile_pool(name="psF", bufs=2, space="PSUM"))
        psPW = ctx.enter_context(tc.tile_pool(name="psPW", bufs=2, space="PSUM"))
        psI = ctx.enter_context(tc.tile_pool(name="psI", bufs=2, space="PSUM"))

        wft = cpool.tile([128, 2, 256], DT.bfloat16)
        wit = cpool.tile([128, 2, V], DT.bfloat16)
        biast = cpool.tile([128, 2], DT.float32)
        for tcn in range(2):
            nc.scalar.dma_start(wft[:, tcn, :], wf_d[tcn])
            nc.scalar.dma_start(wit[:, tcn, :], wi_d[tcn])
        nc.scalar.dma_start(biast[:], bias_d[:])

        # persistent SBUF stores
        xh = cpool.tile([128, 2, 256, S], DT.bfloat16, name="xh", tag="xh")
        ohA = cpool.tile([128, S, 256], DT.bfloat16, name="ohA", tag="ohA")
        ohB = cpool.tile([128, S, 256], DT.bfloat16, name="ohB", tag="ohB")

        # ---- forward DFT ----
        for b in range(bpc):
            for i in range(SEGS):
                s = b * SEGS + i
                xs = xpool.tile([128, 2, CIN], DT.bfloat16)
                nc.scalar.dma_start(xs[:], xt_d[b, i].transpose([1, 0, 2]))
                pf = psF.tile([128, 2, 256], DT.float32)
                for cb in range(2):
                    for tcn in range(2):
                        nc.tensor.matmul(
                            pf[:, cb, :],
                            xs[:, tcn, cb * 128 : (cb + 1) * 128],
                            wft[:, tcn, :],
                            start=(tcn == 0),
                            stop=(tcn == 1),
                            skip_group_check=True,
                        )
                if s % 2 == 0:
                    nc.vector.tensor_copy(xh[:, :, :, s], pf[:])
                else:
                    nc.scalar.copy(xh[:, :, :, s], pf[:])

        # ---- pointwise complex multiply ----
        # jobs per bin k: Re chain (always) into psum col 0, Im chain
        # (k=1..127) into col 1. Drains go into wide 8-row staging tiles
        # (one for ohA rows = Re k, one for ohB rows = Im k / Re 128),
        # flushed by single corner-turn DMAs.
        GRP = 4
        stA = stB = None
        stA_rows = stB_rows = None

        def flush(st, rows, oh):
            if st is None or not rows:
                return
            # partition dim must stay outermost in SBUF APs, so scatter
            # row-by-row: dst [1, S, 256] <- src [S, 1, 256]
            for j, r in enumerate(rows):
                nc.gpsimd.dma_start(oh[r : r + 1], st[:, j : j + 1, :])

        for k in range(NBIN):
            dht = dhpool.tile([128, 2, 2, COUT], DT.bfloat16)
            nc.sync.dma_start(dht[:], dh_d[k])
            has_im = 0 < k < NBIN - 1
            if has_im:
                ng = ngpool.tile([128, 2, S], DT.bfloat16)
                for cb in range(2):
                    nc.vector.tensor_scalar_mul(
                        ng[:, cb, :], xh[:, cb, 128 + k, :], -1.0
                    )
            ps = psPW.tile([S, 2, 256], DT.float32)
            # Re: Xr*Dr + (-Xi)*Di
            n_acc = 4 if has_im else 2
            idx = 0
            for cb in range(2):
                nc.tensor.matmul(
                    ps[:, 0, :], xh[:, cb, k, :], dht[:, cb, 0, :],
                    start=(idx == 0), stop=(idx == n_acc - 1),
                    skip_group_check=True,
                )
                idx += 1
            if has_im:
                for cb in range(2):
                    nc.tensor.matmul(
                        ps[:, 0, :], ng[:, cb, :], dht[:, cb, 1, :],
                        start=False, stop=(idx == n_acc - 1),
                        skip_group_check=True,
                    )
                    idx += 1
                # Im: Xr*Di + Xi*Dr
                idx = 0
                for cb in range(2):
                    nc.tensor.matmul(
                        ps[:, 1, :], xh[:, cb, k, :], dht[:, cb, 1, :],
                        start=(idx == 0), stop=False,
                        skip_group_check=True,
                    )
                    idx += 1
                for cb in range(2):
                    nc.tensor.matmul(
                        ps[:, 1, :], xh[:, cb, 128 + k, :], dht[:, cb, 0, :],
                        start=False, stop=(idx == 3),
                        skip_group_check=True,
                    )
                    idx += 1
            # stage Re row (ohA row k, or ohB row 0 for k=128)
            def stage(dst, src, use_act):
                if use_act:
                    nc.scalar.copy(dst, src)
                else:
                    nc.vector.tensor_copy(dst, src)

            if k < 128:
                if stA is None:
                    stA = stpool.tile([S, GRP, 256], DT.bfloat16)
                    stA_rows = []
                stage(stA[:, len(stA_rows), :], ps[:, 0, :], k % 2)
                stA_rows.append(k)
                if len(stA_rows) == GRP:
                    flush(stA, stA_rows, ohA)
                    stA = None
            else:
                stx = stpool.tile([S, 1, 256], DT.bfloat16)
                stage(stx[:, 0, :], ps[:, 0, :], k % 2)
                flush(stx, [0], ohB)
            # stage Im row (ohB row k)
            if has_im:
                if stB is None:
                    stB = stpool.tile([S, GRP, 256], DT.bfloat16)
                    stB_rows = []
                stage(stB[:, len(stB_rows), :], ps[:, 1, :], (k + 1) % 2)
                stB_rows.append(k)
                if len(stB_rows) == GRP:
                    flush(stB, stB_rows, ohB)
                    stB = None
        flush(stA, stA_rows, ohA)
        flush(stB, stB_rows, ohB)

        # ---- inverse DFT + bias ----
        for s in range(S):
            b, i = divmod(s, SEGS)
            pv = psI.tile([128, 2, V], DT.float32)
            for ot in range(2):
                nc.tensor.matmul(
                    pv[:, ot, :], ohA[:, s, ot * 128 : (ot + 1) * 128], wit[:, 0, :],
                    start=True, stop=False, skip_group_check=True,
                )
                nc.tensor.matmul(
                    pv[:, ot, :], ohB[:, s, ot * 128 : (ot + 1) * 128], wit[:, 1, :],
                    start=False, stop=True, skip_group_check=True,
                )
            yo = ypool.tile([128, 2, V], DT.float32)
            for ot in range(2):
                if s % 2 == 0:
                    nc.vector.tensor_scalar_add(
                        yo[:, ot, :], pv[:, ot, :], biast[:, ot : ot + 1]
                    )
                else:
                    nc.scalar.add(yo[:, ot, :], pv[:, ot, :], biast[:, ot : ot + 1])
            nc.scalar.dma_start(
                y_d[b, :, :, i * V : (i + 1) * V].transpose([1, 0, 2]), yo[:]
            )

    nc.compile()
    return nc


def host_inputs(input, weight, P, bias):
    """Host-side staging: xt segments (transposed, bf16) + spectra consts."""
    D = build_dense_kernel(weight, P)
    Wf, WI, Dh = build_consts(D)
    wf = np.ascontiguousarray(Wf.reshape(2, 128, 256)).astype(BF)
    wi = np.ascontiguousarray(WI.reshape(2, 128, V)).astype(BF)
    # Dh [129, 2ri, C, O] -> dh[k, cp, cb, ri, o]
    dh = np.ascontiguousarray(
        Dh.reshape(NBIN, 2, 2, 128, COUT).transpose(0, 3, 2, 1, 4)
    ).astype(BF)
    bias2 = np.ascontiguousarray(np.asarray(bias, np.float32).reshape(2, 128).T)
    xpad = np.zeros((input.shape[0], CIN, LPAD), np.float32)
    xpad[:, :, PAD : PAD + L] = input
    xpad = xpad.astype(BF)
    idx = V * np.arange(SEGS)[:, None] + np.arange(F)[None, :]
    segs = xpad[:, :, idx]  # [B, C, SEGS, F]
    xt = np.ascontiguousarray(segs.transpose(0, 2, 3, 1))  # [B, SEGS, F, C]
    xt = xt.reshape(input.shape[0], SEGS, 2, 128, CIN)
    return xt, dh, wf, wi, bias2


def make_in_maps(inputs):
    xt, dh, wf, wi, bias2 = host_inputs(
        np.ascontiguousarray(inputs["input"], np.float32),
        inputs["weight"],
        inputs["P"],
        inputs["bias"],
    )
    return [
        {
            "xt": np.ascontiguousarray(xt[i * BPC : (i + 1) * BPC]),
            "dh": dh,
            "wf": wf,
            "wi": wi,
            "bias": bias2,
        }
        for i in range(NCORES)
    ]


def kernel(input, weight, P, bias):
    if "nc" not in _nc_cache:
        _nc_cache["nc"] = build_nc()
    nc = _nc_cache["nc"]
    in_maps = make_in_maps(
        {"input": input, "weight": weight, "P": P, "bias": bias}
    )
    res = run_bass_kernel_spmd(nc, in_maps, core_ids=list(range(NCORES)))
    out = np.concatenate(
        [r["y"].reshape(BPC, COUT, YL) for r in res.results], axis=0
    )
    return np.ascontiguousarray(out[:, :, :TOUT])
